# revision 27
# baseline (speedup 1.0000x reference)
"""MultiHeadAttention TRN2 Bass kernel (v3).

Full-input contract: kernel(**inputs) takes the unsharded tensors from
setup_inputs() and returns the full [4, 2048, 512] output.

Sharding: 8 cores = 4 batches x 2 query-halves. Each core computes its own
[1024, 512] slice of the output for one batch over all 8 heads, so the
gather is a pure concatenation (no collectives, no all-reduce).

v3 structure (vs the phase-sequential baseline):
  - All-bf16 on the PE (transposes 1.0 cyc/row, half the DVE copy bytes).
  - Software-pipelined projections: only W / x_q / x_k transposes and the
    head-pair-0 Q/K projection run before attention.  The V projection,
    x_v transposes, and head pairs 1-3's Q/K projections are emitted as
    "filler" work interleaved into the attention instruction stream, so
    the PE always has dependency-free work queued while exp drains score
    PSUM tiles -- the Tensor engine's clock ramps with sustained stall-free
    execution (measured 427ns vs 216ns per 512-row matmul), so filler
    directly buys clock.
  - exp in [128, 1024] chunks (half the ACT instruction overhead).
  - Per-head-pair epilogue (rowsum broadcast matmul -> 128-lane reciprocal
    -> normalize) runs inside the attention loop; the broadcast PSUM tile
    shares the score pool and its ~7us reciprocal is absorbed by filler.
"""
import contextlib

import numpy as np

import bass_rust
import concourse.bass as bass
import concourse.mybir as mybir
import concourse.tile as tile
from concourse.bass_utils import run_bass_kernel_spmd
from concourse.masks import make_identity
from concourse.tile import add_dep_helper

F32 = mybir.dt.float32
F32R = mybir.dt.float32r
BF16 = mybir.dt.bfloat16

B, S, D_MODEL = 4, 2048, 512
NUM_HEADS = 8
HEAD_DIM = 64
SQ = S // 2  # queries per core
N_CORES = 8
SCALE = 1.0 / 8.0  # 1/sqrt(HEAD_DIM)
KTILES = S // 128  # 16
H2 = NUM_HEADS // 2  # head pairs = dout tiles of 128
QC = SQ // 512  # 512-wide query chunks per score tile

_split_ctr = [0]


def split_waits(nc, max_waits: int = 1):
    """walrus codegen rejects instructions carrying >1 sync wait; move the
    extras onto standalone EventSemaphore instructions on the same engine."""
    for f in nc.m.functions:
        for blk in f.blocks:
            new_insts = []
            changed = False
            for inst in blk.instructions:
                si = inst.sync_info
                if si is not None and si.on_wait and len(si.on_wait) > max_waits:
                    waits = list(si.on_wait)
                    extra, keep = waits[:-max_waits], waits[-max_waits:]
                    for w in extra:
                        _split_ctr[0] += 1
                        ev = mybir.InstEventSemaphore(
                            name=f"I-wsplit-{_split_ctr[0]}", ins=[], outs=[]
                        )
                        ev.engine = inst.engine
                        ev.sync_info = bass_rust.SyncInfo(on_wait=[w], on_update=[])
                        new_insts.append(ev)
                    inst.sync_info = bass_rust.SyncInfo(
                        on_wait=keep, on_update=list(si.on_update)
                    )
                    changed = True
                new_insts.append(inst)
            if changed:
                blk.instructions = new_insts


def build_mha():
    nc = bass.Bass("TRN2", target_bir_lowering=False, debug=False, num_devices=1)

    qd = nc.declare_dram_parameter("q", [SQ, D_MODEL], F32, isOutput=False).ap()
    kd = nc.declare_dram_parameter("k", [S, D_MODEL], F32, isOutput=False).ap()
    vd = nc.declare_dram_parameter("v", [S, D_MODEL], F32, isOutput=False).ap()
    wts = {
        n: nc.declare_dram_parameter(n, [D_MODEL, D_MODEL], F32, isOutput=False).ap()
        for n in ("wq", "wk", "wv", "wo")
    }
    bias = {
        n: nc.declare_dram_parameter(n, [D_MODEL], F32, isOutput=False).ap()
        for n in ("bq", "bk", "bv", "bo")
    }
    outd = nc.declare_dram_parameter("out", [SQ, D_MODEL], F32, isOutput=True).ap()

    with tile.TileContext(nc) as tc, contextlib.ExitStack() as top:
        consts = top.enter_context(tc.tile_pool(name="consts", bufs=1))
        wt_pool = top.enter_context(tc.tile_pool(name="wt", bufs=1))
        qkt_pool = top.enter_context(tc.tile_pool(name="qkt", bufs=1))
        xqk_pool = top.enter_context(tc.tile_pool(name="xqk", bufs=1))
        proj_out = top.enter_context(tc.tile_pool(name="proj_out", bufs=1))
        epilog = top.enter_context(tc.tile_pool(name="epilog", bufs=1))
        ld_pool = top.enter_context(tc.tile_pool(name="ld", bufs=2))
        ldc_pool = top.enter_context(tc.tile_pool(name="ldc", bufs=2))
        xtv_pool = top.enter_context(tc.tile_pool(name="xtv", bufs=1))
        # score psum: 2 tiles x 2 banks, top level (shared with the epilogue
        # broadcast tile)
        ps_s = top.enter_context(tc.tile_pool(name="ps_s", bufs=2, space="PSUM"))

        # ---- constants
        identb = consts.tile([128, 128], BF16)
        make_identity(nc, identb)
        bqt = consts.tile([128, 4], F32)
        bkt = consts.tile([128, 4], F32)
        for t_, name in ((bqt, "bq"), (bkt, "bk")):
            nc.gpsimd.dma_start(
                out=t_, in_=bias[name].rearrange("(c p) -> p c", p=128)
            )
        bvb = consts.tile([128, D_MODEL], F32)
        bob = consts.tile([128, D_MODEL], F32)
        for t_, name in ((bvb, "bv"), (bob, "bo")):
            src = bias[name]
            nc.gpsimd.dma_start(
                out=t_,
                in_=bass.AP(tensor=src.tensor, offset=src.offset, ap=[[0, 128], [1, D_MODEL]]),
            )
        import ml_dtypes as _mld
        # selector for the reciprocal broadcast: contraction partitions 0 and
        # 32 carry the two heads' reciprocal rows (32 is a legal AP base
        # partition; 1 is not), rows 1-31 are zero.
        eud_np = np.zeros((33, 128), _mld.bfloat16)
        eud_np[0, 0:HEAD_DIM] = 1.0
        eud_np[32, HEAD_DIM:128] = 1.0
        eud_dram = nc.inline_tensor(eud_np, name="eud_const")
        e2 = consts.tile([33, 128], BF16)
        nc.gpsimd.dma_start(out=e2, in_=eud_dram.ap())

        # ---- long-lived tiles
        QT = [
            qkt_pool.tile([128, SQ], BF16, name=f"qt_{t}", tag=f"qt_{t}")
            for t in range(H2)
        ]
        KT = [
            qkt_pool.tile([128, S], BF16, name=f"kt_{t}", tag=f"kt_{t}")
            for t in range(H2)
        ]
        WT = {
            n: [
                wt_pool.tile([128, D_MODEL], BF16, name=f"wt_{n}_{dc}", tag=f"wt_{n}_{dc}")
                for dc in range(4)
            ]
            for n in ("wq", "wk", "wv", "wo")
        }
        # persistent x^T for q and k (per 512-token chunk)
        XTQ = [
            xqk_pool.tile([128, 4, D_MODEL], BF16, name=f"xtq_{c}", tag=f"xtq_{c}")
            for c in range(SQ // 512)
        ]
        XTK = [
            xqk_pool.tile([128, 4, D_MODEL], BF16, name=f"xtk_{c}", tag=f"xtk_{c}")
            for c in range(S // 512)
        ]
        V = [
            proj_out.tile([128, NUM_HEADS, HEAD_DIM + 1], BF16, name=f"v_{sc}", tag=f"v_{sc}")
            for sc in range(KTILES)
        ]
        OMT = [
            epilog.tile([128, SQ], BF16, name=f"omt_{t}", tag=f"omt_{t}")
            for t in range(H2)
        ]
        # out-projection partial sums (pairs 0-2 accumulated as filler work)
        OACC = [
            epilog.tile([128, D_MODEL], BF16, name=f"oacc_{sq}", tag=f"oacc_{sq}")
            for sq in range(SQ // 128)
        ]

        pe_chain = [None]

        def chain(bi):
            if pe_chain[0] is not None:
                add_dep_helper(bi.ins, pe_chain[0].ins, reason="pe-order")
            pe_chain[0] = bi

        def load_chunk(src_ap, s0, nt, cast_fn, nm):
            """DMA [nt*128, 512] rows from src into sbuf and cast to bf16."""
            x_nat = ld_pool.tile([128, 4, D_MODEL], F32, name=f"xn_{nm}", tag="ldraw")
            nc.sync.dma_start(
                out=x_nat[:, 0:nt, :],
                in_=src_ap[s0 : s0 + nt * 128, :].rearrange("(c p) m -> p c m", p=128),
            )
            x_bf = ldc_pool.tile([128, 4, D_MODEL], BF16, name=f"xb_{nm}", tag="ldc")
            cast_fn(x_bf[:, 0:nt, :], x_nat[:, 0:nt, :])
            return x_bf

        def transpose_group(pt_dst, x_bf, st):
            """4 PE transposes: x_bf's st-th 128-token block -> pt psum."""
            for dc in range(4):
                chain(
                    nc.tensor.transpose(
                        pt_dst[:, dc * 128 : (dc + 1) * 128],
                        x_bf[:, st, dc * 128 : (dc + 1) * 128],
                        identb,
                    )
                )

        def qk_proj_item(t, wname, bt, dst, xt_c, c, psum_pool, tg):
            """One 4-matmul projection group: pair t's slice of chunk c."""
            pj = psum_pool.tile([128, 512], F32, name=f"pj_{tg}", tag=tg)
            for dc in range(4):
                chain(
                    nc.tensor.matmul(
                        pj,
                        WT[wname][dc][:, t * 128 : (t + 1) * 128],
                        xt_c[:, dc, :],
                        start=(dc == 0),
                        stop=(dc == 3),
                    )
                )
            nc.vector.tensor_scalar_add(
                dst[t][:, c * 512 : (c + 1) * 512], pj, bt[:, t : t + 1]
            )

        # ============ startup: W transposes, x_q/x_k transposes, pair 0 =====
        with (
            tc.tile_pool(name="pp0", bufs=2, space="PSUM") as pp0,
            tc.tile_pool(name="pt0", bufs=2, space="PSUM") as pt0,
        ):
            for name, wd in wts.items():
                w_bf = load_chunk(wd, 0, 4, nc.scalar.copy, f"w_{name}")
                for oc in range(4):
                    pt = pt0.tile([128, D_MODEL], BF16, name="pt0t", tag="pt0")
                    transpose_group(pt, w_bf, oc)
                    for dc in range(4):
                        nc.vector.tensor_copy(
                            WT[name][dc][:, oc * 128 : (oc + 1) * 128],
                            pt[:, dc * 128 : (dc + 1) * 128],
                        )

            for src_ap, xt_list, nm in ((qd, XTQ, "q"), (kd, XTK, "k")):
                for c, xt_c in enumerate(xt_list):
                    x_bf = load_chunk(src_ap, c * 512, 4, nc.scalar.copy, f"{nm}{c}")
                    for st in range(4):
                        pt = pt0.tile([128, D_MODEL], BF16, name="pt0t", tag="pt0")
                        transpose_group(pt, x_bf, st)
                        nc.vector.tensor_copy(
                            xt_c[:, :, st * 128 : (st + 1) * 128],
                            pt.rearrange("p (c f) -> p c f", c=4),
                        )

            # pair 0 Q/K projection runs before attention
            for c, xt_c in enumerate(XTQ):
                qk_proj_item(0, "wq", bqt, QT, xt_c, c, pp0, "pj0")
            for c, xt_c in enumerate(XTK):
                qk_proj_item(0, "wk", bkt, KT, xt_c, c, pp0, "pj0")

        # preset the V ones-columns
        for sc in range(KTILES):
            nc.vector.memset(V[sc][:, :, HEAD_DIM : HEAD_DIM + 1], 1.0)
        # x_v loads: DMA + Pool casts emitted now; consumed by filler
        XBV = [
            load_chunk(vd, c * 512, 4, nc.gpsimd.tensor_copy, f"v{c}") for c in range(S // 512)
        ]

        # ============ attention with projection filler ======================
        with (
            tc.tile_pool(name="ehpool", bufs=2) as ehpool,
            tc.tile_pool(name="oupool", bufs=2) as oupool,
            tc.tile_pool(name="rrpool", bufs=2) as rrpool,
            tc.tile_pool(name="rsbpool", bufs=1) as rsbpool,
            tc.tile_pool(name="ps_o", bufs=1, space="PSUM") as ps_o,
            tc.tile_pool(name="pp", bufs=1, space="PSUM") as pp,
            tc.tile_pool(name="pt", bufs=1, space="PSUM") as pt_pool,
        ):
            KB = 2
            pe_chain[0] = None  # head-0 scores may overlap the startup tail

            # ---- filler queue: V-projection, pairs 1-3 Q/K projections, and
            # the pair 0-2 out-projection partials.  Entries are
            # (min_head, closure): a closure may only be emitted at head >=
            # min_head (its tile dependencies exist by then).  Keeping the PE
            # fed with dependency-free filler through ALL heads holds the
            # Tensor clock at full speed (stalls drop it 2x and it stays
            # dropped).
            filler = []
            vt_state = [None]

            def vt_item(c, st):
                def run():
                    if st == 0:
                        vt_state[0] = xtv_pool.tile(
                            [128, 4, D_MODEL], BF16, name=f"xtv_{c}", tag="xtv"
                        )
                    xtv = vt_state[0]
                    pt = pt_pool.tile([128, D_MODEL], BF16, name="ptv", tag="ptv")
                    transpose_group(pt, XBV[c], st)
                    nc.vector.tensor_copy(
                        xtv[:, :, st * 128 : (st + 1) * 128],
                        pt.rearrange("p (c f) -> p c f", c=4),
                    )
                return run

            def vp_item(c, st):
                def run():
                    xtv = vt_state[0]
                    sc = c * 4 + st
                    pj = pp.tile([128, 512], F32, name="pjv", tag="ppj")
                    for dc in range(4):
                        chain(
                            nc.tensor.matmul(
                                pj,
                                xtv[:, dc, st * 128 : (st + 1) * 128],
                                WT["wv"][dc],
                                start=(dc == 0),
                                stop=(dc == 3),
                            )
                        )
                    pj3 = pj.rearrange("p (h d) -> p h d", h=NUM_HEADS)
                    nc.vector.tensor_add(
                        V[sc][:, :, 0:HEAD_DIM],
                        pj3,
                        bvb.rearrange("p (h d) -> p h d", h=NUM_HEADS),
                    )
                return run

            def qk_item(t, wname, bt, dst, xt_c, c):
                def run():
                    qk_proj_item(t, wname, bt, dst, xt_c, c, pp, "ppj")
                return run

            def op_item(t, sq):
                def run():
                    pj = pp.tile([128, D_MODEL], F32, name="pjo", tag="ppj")
                    chain(
                        nc.tensor.matmul(
                            pj,
                            OMT[t][:, sq * 128 : (sq + 1) * 128],
                            WT["wo"][t],
                            start=True,
                            stop=True,
                        )
                    )
                    if t == 0:
                        nc.vector.tensor_add(OACC[sq], pj, bob)
                    else:
                        nc.vector.tensor_add(OACC[sq], OACC[sq], pj)
                return run

            for c in range(S // 512):
                for st in range(4):
                    filler.append((0, vt_item(c, st)))
                    filler.append((0, vp_item(c, st)))
            for t in range(1, H2):
                mh = t - 1  # pair t's projection may start at head t-1
                for c, xt_c in enumerate(XTQ):
                    filler.append((mh, qk_item(t, "wq", bqt, QT, xt_c, c)))
                for c, xt_c in enumerate(XTK):
                    filler.append((mh, qk_item(t, "wk", bkt, KT, xt_c, c)))
            # out-projection partials: OMT[t] exists after epilogue_pair(t),
            # emitted at the end of head 2t+2 -> gate at head 2t+3.
            for t in range(H2 - 1):
                for sq in range(SQ // 128):
                    filler.append((2 * t + 3 + (sq // 4), op_item(t, sq)))

            # pops per kb batch, by head
            POPS = [3, 3, 1, 1, 1, 1, 1, 1]

            OU = {}
            RR = {}
            PO = {}
            deferred = []

            def emit_av_batch(ph, peh, kb):
                if kb == 0:
                    PO[ph] = ps_o.tile([HEAD_DIM + 1, SQ], F32, name=f"po_{ph}", tag="po")
                ppo = PO[ph]
                for j in range(KB):
                    kc = KB * kb + j
                    for qc in range(QC):
                        sl = slice(qc * 512, (qc + 1) * 512)
                        chain(
                            nc.tensor.matmul(
                                ppo[:, sl],
                                V[kc][:, ph, :],
                                peh[:, kc, sl],
                                start=(kc == 0),
                                stop=(kc == KTILES - 1),
                            )
                        )

            def epilogue_recip(t):
                # reciprocal runs in SBUF (holds no PSUM; a DVE reciprocal is
                # ~7 cyc/elem = ~7us per pair).  The PE-side broadcast is
                # DEFERRED one head: the in-order PE queue would stall behind
                # a matmul whose rcp input is still being computed.
                rcp = rsbpool.tile([33, SQ], BF16, name=f"rcp_{t}", tag="rsb")
                with nc.allow_low_precision(reason="bf16 softmax denominators, ~0.4% on a 2e-2 budget"):
                    nc.vector.reciprocal(rcp, RR[t])
                deferred.append((t, rcp))

            def epilogue_bcast(t, rcp):
                pr = ps_s.tile([128, SQ], F32, name=f"pr_{t}", tag="pscore")
                for qc in range(QC):
                    sl = slice(qc * 512, (qc + 1) * 512)
                    chain(nc.tensor.matmul(pr[:, sl], e2, rcp[:, sl], start=True, stop=True))
                nc.vector.tensor_mul(OMT[t], OU[t], pr)

            def finalize(ph):
                ppo = PO.pop(ph)
                t, half = ph // 2, ph % 2
                if half == 0:
                    OU[t] = oupool.tile([128, SQ], BF16, name=f"ou_{t}", tag="ou")
                    RR[t] = rrpool.tile([33, SQ], BF16, name=f"rrp_{t}", tag="rr")
                    nc.vector.memset(RR[t], 1.0)
                nc.vector.tensor_copy(
                    OU[t][half * HEAD_DIM : (half + 1) * HEAD_DIM, :],
                    ppo[0:HEAD_DIM, :],
                )
                nc.vector.tensor_copy(
                    RR[t][32 * half : 32 * half + 1, :],
                    ppo[HEAD_DIM : HEAD_DIM + 1, :],
                )
                if half == 1:
                    epilogue_recip(t)

            prev = None
            for h in range(NUM_HEADS):
                eh = ehpool.tile([128, KTILES, SQ], BF16, name=f"eh_{h}", tag="eh")
                t, half = h // 2, h % 2
                if deferred:
                    epilogue_bcast(*deferred.pop(0))
                for kb in range(KTILES // KB):
                    pss = []
                    for j in range(KB):
                        kc = KB * kb + j
                        pscore = ps_s.tile([128, SQ], F32, name="psc", tag="pscore")
                        for qc in range(QC):
                            sl = slice(qc * 512, (qc + 1) * 512)
                            chain(
                                nc.tensor.matmul(
                                    pscore[:, sl],
                                    KT[t][
                                        half * HEAD_DIM : (half + 1) * HEAD_DIM,
                                        kc * 128 : (kc + 1) * 128,
                                    ],
                                    QT[t][
                                        half * HEAD_DIM : (half + 1) * HEAD_DIM, sl
                                    ],
                                    start=True,
                                    stop=True,
                                )
                            )
                        pss.append((kc, pscore))
                    for kc, pscore in pss:
                        nc.scalar.activation(
                            eh[:, kc, :],
                            pscore,
                            mybir.ActivationFunctionType.Exp,
                            scale=SCALE,
                        )
                    if prev is not None:
                        emit_av_batch(prev[0], prev[1], kb)
                    for _ in range(POPS[h]):
                        if filler and filler[0][0] <= h:
                            filler.pop(0)[1]()
                if prev is not None:
                    finalize(prev[0])
                prev = (h, eh)
            for kb in range(KTILES // KB):
                emit_av_batch(prev[0], prev[1], kb)
                while filler and filler[0][0] <= NUM_HEADS:
                    filler.pop(0)[1]()
                    break
            finalize(prev[0])
            while deferred:
                epilogue_bcast(*deferred.pop(0))
            while filler:
                filler.pop(0)[1]()

            # tail: pair 3's out-projection contribution + store
            with tc.tile_pool(name="outsb", bufs=2) as outsb:
                for sq in range(SQ // 128):
                    pj = pp.tile([128, D_MODEL], F32, name="pjt", tag="ppj")
                    chain(
                        nc.tensor.matmul(
                            pj,
                            OMT[3][:, sq * 128 : (sq + 1) * 128],
                            WT["wo"][3],
                            start=True,
                            stop=True,
                        )
                    )
                    ot = outsb.tile([128, D_MODEL], F32, name="ot", tag="ot")
                    nc.vector.tensor_add(ot, pj, OACC[sq])
                    nc.sync.dma_start(out=outd[sq * 128 : (sq + 1) * 128, :], in_=ot)

    split_waits(nc)
    return nc


_cached_nc = None


def _get_nc():
    global _cached_nc
    if _cached_nc is None:
        _cached_nc = build_mha()
    return _cached_nc


def kernel(q, k, v, mask, Wq, bq, Wk, bk, Wv, bv, Wo, bo, **_unused):
    q = np.asarray(q, dtype=np.float32)
    k = np.asarray(k, dtype=np.float32)
    v = np.asarray(v, dtype=np.float32)
    weights = {
        "wq": np.ascontiguousarray(np.asarray(Wq, np.float32)),
        "wk": np.ascontiguousarray(np.asarray(Wk, np.float32)),
        "wv": np.ascontiguousarray(np.asarray(Wv, np.float32)),
        "wo": np.ascontiguousarray(np.asarray(Wo, np.float32)),
        "bq": np.ascontiguousarray(np.asarray(bq, np.float32)),
        "bk": np.ascontiguousarray(np.asarray(bk, np.float32)),
        "bv": np.ascontiguousarray(np.asarray(bv, np.float32)),
        "bo": np.ascontiguousarray(np.asarray(bo, np.float32)),
    }
    in_maps = []
    for core in range(N_CORES):
        b, qh = core // 2, core % 2
        in_maps.append(
            {
                "q": np.ascontiguousarray(q[b, qh * SQ : (qh + 1) * SQ, :]),
                "k": np.ascontiguousarray(k[b]),
                "v": np.ascontiguousarray(v[b]),
                **weights,
            }
        )
    nc = _get_nc()
    res = run_bass_kernel_spmd(nc, in_maps, list(range(N_CORES)))
    out = np.empty((B, S, D_MODEL), dtype=np.float32)
    for core in range(N_CORES):
        b, qh = core // 2, core % 2
        out[b, qh * SQ : (qh + 1) * SQ, :] = res.results[core]["out"]
    return out


# revision 29
# speedup vs baseline: 1.0439x; 1.0439x over previous
"""MultiHeadAttention TRN2 Bass kernel (v3).

Full-input contract: kernel(**inputs) takes the unsharded tensors from
setup_inputs() and returns the full [4, 2048, 512] output.

Sharding: 8 cores = 4 batches x 2 query-halves. Each core computes its own
[1024, 512] slice of the output for one batch over all 8 heads, so the
gather is a pure concatenation (no collectives, no all-reduce).

v3 structure (vs the phase-sequential baseline):
  - All-bf16 on the PE (transposes 1.0 cyc/row, half the DVE copy bytes).
  - Software-pipelined projections: only W / x_q / x_k transposes and the
    head-pair-0 Q/K projection run before attention.  The V projection,
    x_v transposes, and head pairs 1-3's Q/K projections are emitted as
    "filler" work interleaved into the attention instruction stream, so
    the PE always has dependency-free work queued while exp drains score
    PSUM tiles -- the Tensor engine's clock ramps with sustained stall-free
    execution (measured 427ns vs 216ns per 512-row matmul), so filler
    directly buys clock.
  - exp in [128, 1024] chunks (half the ACT instruction overhead).
  - Per-head-pair epilogue (rowsum broadcast matmul -> 128-lane reciprocal
    -> normalize) runs inside the attention loop; the broadcast PSUM tile
    shares the score pool and its ~7us reciprocal is absorbed by filler.
"""
import contextlib

import numpy as np

import bass_rust
import concourse.bass as bass
import concourse.mybir as mybir
import concourse.tile as tile
from concourse.bass_utils import run_bass_kernel_spmd
from concourse.masks import make_identity
from concourse.tile import add_dep_helper

F32 = mybir.dt.float32
F32R = mybir.dt.float32r
BF16 = mybir.dt.bfloat16

B, S, D_MODEL = 4, 2048, 512
NUM_HEADS = 8
HEAD_DIM = 64
SQ = S // 2  # queries per core
N_CORES = 8
SCALE = 1.0 / 8.0  # 1/sqrt(HEAD_DIM)
KTILES = S // 128  # 16
H2 = NUM_HEADS // 2  # head pairs = dout tiles of 128
QC = SQ // 512  # 512-wide query chunks per score tile

_split_ctr = [0]


def split_waits(nc, max_waits: int = 1):
    """walrus codegen rejects instructions carrying >1 sync wait; move the
    extras onto standalone EventSemaphore instructions on the same engine."""
    for f in nc.m.functions:
        for blk in f.blocks:
            new_insts = []
            changed = False
            for inst in blk.instructions:
                si = inst.sync_info
                if si is not None and si.on_wait and len(si.on_wait) > max_waits:
                    waits = list(si.on_wait)
                    extra, keep = waits[:-max_waits], waits[-max_waits:]
                    for w in extra:
                        _split_ctr[0] += 1
                        ev = mybir.InstEventSemaphore(
                            name=f"I-wsplit-{_split_ctr[0]}", ins=[], outs=[]
                        )
                        ev.engine = inst.engine
                        ev.sync_info = bass_rust.SyncInfo(on_wait=[w], on_update=[])
                        new_insts.append(ev)
                    inst.sync_info = bass_rust.SyncInfo(
                        on_wait=keep, on_update=list(si.on_update)
                    )
                    changed = True
                new_insts.append(inst)
            if changed:
                blk.instructions = new_insts


def build_mha():
    nc = bass.Bass("TRN2", target_bir_lowering=False, debug=False, num_devices=1)

    qd = nc.declare_dram_parameter("q", [SQ, D_MODEL], F32, isOutput=False).ap()
    kd = nc.declare_dram_parameter("k", [S, D_MODEL], F32, isOutput=False).ap()
    vd = nc.declare_dram_parameter("v", [S, D_MODEL], F32, isOutput=False).ap()
    wts = {
        n: nc.declare_dram_parameter(n, [D_MODEL, D_MODEL], F32, isOutput=False).ap()
        for n in ("wq", "wk", "wv", "wo")
    }
    bias = {
        n: nc.declare_dram_parameter(n, [D_MODEL], F32, isOutput=False).ap()
        for n in ("bq", "bk", "bv", "bo")
    }
    outd = nc.declare_dram_parameter("out", [SQ, D_MODEL], F32, isOutput=True).ap()

    with tile.TileContext(nc) as tc, contextlib.ExitStack() as top:
        consts = top.enter_context(tc.tile_pool(name="consts", bufs=1))
        wt_pool = top.enter_context(tc.tile_pool(name="wt", bufs=1))
        qkt_pool = top.enter_context(tc.tile_pool(name="qkt", bufs=1))
        xqk_pool = top.enter_context(tc.tile_pool(name="xqk", bufs=1))
        proj_out = top.enter_context(tc.tile_pool(name="proj_out", bufs=1))
        epilog = top.enter_context(tc.tile_pool(name="epilog", bufs=1))
        ld_pool = top.enter_context(tc.tile_pool(name="ld", bufs=2))
        ldc_pool = top.enter_context(tc.tile_pool(name="ldc", bufs=2))
        xtv_pool = top.enter_context(tc.tile_pool(name="xtv", bufs=1))
        # score psum: 2 tiles x 2 banks, top level (shared with the epilogue
        # broadcast tile)
        ps_s = top.enter_context(tc.tile_pool(name="ps_s", bufs=2, space="PSUM"))

        # ---- constants
        identb = consts.tile([128, 128], BF16)
        make_identity(nc, identb)
        bqt = consts.tile([128, 4], F32)
        bkt = consts.tile([128, 4], F32)
        for t_, name in ((bqt, "bq"), (bkt, "bk")):
            nc.gpsimd.dma_start(
                out=t_, in_=bias[name].rearrange("(c p) -> p c", p=128)
            )
        bvb = consts.tile([128, D_MODEL], F32)
        bob = consts.tile([128, D_MODEL], F32)
        for t_, name in ((bvb, "bv"), (bob, "bo")):
            src = bias[name]
            nc.gpsimd.dma_start(
                out=t_,
                in_=bass.AP(tensor=src.tensor, offset=src.offset, ap=[[0, 128], [1, D_MODEL]]),
            )
        import ml_dtypes as _mld
        # selector for the reciprocal broadcast: contraction partitions 0 and
        # 32 carry the two heads' reciprocal rows (32 is a legal AP base
        # partition; 1 is not), rows 1-31 are zero.
        eud_np = np.zeros((33, 128), _mld.bfloat16)
        eud_np[0, 0:HEAD_DIM] = 1.0
        eud_np[32, HEAD_DIM:128] = 1.0
        eud_dram = nc.inline_tensor(eud_np, name="eud_const")
        e2 = consts.tile([33, 128], BF16)
        nc.gpsimd.dma_start(out=e2, in_=eud_dram.ap())

        # ---- long-lived tiles
        QT = [
            qkt_pool.tile([128, SQ], BF16, name=f"qt_{t}", tag=f"qt_{t}")
            for t in range(H2)
        ]
        KT = [
            qkt_pool.tile([128, S], BF16, name=f"kt_{t}", tag=f"kt_{t}")
            for t in range(H2)
        ]
        WT = {
            n: [
                wt_pool.tile([128, D_MODEL], BF16, name=f"wt_{n}_{dc}", tag=f"wt_{n}_{dc}")
                for dc in range(4)
            ]
            for n in ("wq", "wk", "wv", "wo")
        }
        # persistent x^T for q and k (per 512-token chunk)
        XTQ = [
            xqk_pool.tile([128, 4, D_MODEL], BF16, name=f"xtq_{c}", tag=f"xtq_{c}")
            for c in range(SQ // 512)
        ]
        XTK = [
            xqk_pool.tile([128, 4, D_MODEL], BF16, name=f"xtk_{c}", tag=f"xtk_{c}")
            for c in range(S // 512)
        ]
        V = [
            proj_out.tile([128, NUM_HEADS, HEAD_DIM + 1], BF16, name=f"v_{sc}", tag=f"v_{sc}")
            for sc in range(KTILES)
        ]
        OMT = [
            epilog.tile([128, SQ], BF16, name=f"omt_{t}", tag=f"omt_{t}")
            for t in range(H2)
        ]
        # out-projection partial sums (pairs 0-2 accumulated as filler work)
        OACC = [
            epilog.tile([128, D_MODEL], BF16, name=f"oacc_{sq}", tag=f"oacc_{sq}")
            for sq in range(SQ // 128)
        ]

        pe_chain = [None]

        def chain(bi):
            if pe_chain[0] is not None:
                add_dep_helper(bi.ins, pe_chain[0].ins, reason="pe-order")
            pe_chain[0] = bi

        def load_chunk(src_ap, s0, nt, cast_fn, nm):
            """DMA [nt*128, 512] rows from src into sbuf and cast to bf16."""
            x_nat = ld_pool.tile([128, 4, D_MODEL], F32, name=f"xn_{nm}", tag="ldraw")
            nc.sync.dma_start(
                out=x_nat[:, 0:nt, :],
                in_=src_ap[s0 : s0 + nt * 128, :].rearrange("(c p) m -> p c m", p=128),
            )
            x_bf = ldc_pool.tile([128, 4, D_MODEL], BF16, name=f"xb_{nm}", tag="ldc")
            cast_fn(x_bf[:, 0:nt, :], x_nat[:, 0:nt, :])
            return x_bf

        def transpose_group(pt_dst, x_bf, st):
            """4 PE transposes: x_bf's st-th 128-token block -> pt psum."""
            for dc in range(4):
                chain(
                    nc.tensor.transpose(
                        pt_dst[:, dc * 128 : (dc + 1) * 128],
                        x_bf[:, st, dc * 128 : (dc + 1) * 128],
                        identb,
                    )
                )

        def qk_proj_item(t, wname, bt, dst, xt_c, c, psum_pool, tg):
            """One 4-matmul projection group: pair t's slice of chunk c."""
            pj = psum_pool.tile([128, 512], F32, name=f"pj_{tg}", tag=tg)
            for dc in range(4):
                chain(
                    nc.tensor.matmul(
                        pj,
                        WT[wname][dc][:, t * 128 : (t + 1) * 128],
                        xt_c[:, dc, :],
                        start=(dc == 0),
                        stop=(dc == 3),
                    )
                )
            nc.vector.tensor_scalar_add(
                dst[t][:, c * 512 : (c + 1) * 512], pj, bt[:, t : t + 1]
            )

        # ============ startup: W transposes, x_q/x_k transposes, pair 0 =====
        with (
            tc.tile_pool(name="pp0", bufs=2, space="PSUM") as pp0,
            tc.tile_pool(name="pt0", bufs=2, space="PSUM") as pt0,
        ):
            for name, wd in wts.items():
                w_bf = load_chunk(wd, 0, 4, nc.scalar.copy, f"w_{name}")
                for oc in range(4):
                    pt = pt0.tile([128, D_MODEL], BF16, name="pt0t", tag="pt0")
                    transpose_group(pt, w_bf, oc)
                    for dc in range(4):
                        nc.vector.tensor_copy(
                            WT[name][dc][:, oc * 128 : (oc + 1) * 128],
                            pt[:, dc * 128 : (dc + 1) * 128],
                        )

            for src_ap, xt_list, nm in ((qd, XTQ, "q"), (kd, XTK, "k")):
                for c, xt_c in enumerate(xt_list):
                    x_bf = load_chunk(src_ap, c * 512, 4, nc.scalar.copy, f"{nm}{c}")
                    for st in range(4):
                        pt = pt0.tile([128, D_MODEL], BF16, name="pt0t", tag="pt0")
                        transpose_group(pt, x_bf, st)
                        nc.vector.tensor_copy(
                            xt_c[:, :, st * 128 : (st + 1) * 128],
                            pt.rearrange("p (c f) -> p c f", c=4),
                        )

            # pair 0 Q/K projection runs before attention
            for c, xt_c in enumerate(XTQ):
                qk_proj_item(0, "wq", bqt, QT, xt_c, c, pp0, "pj0")
            for c, xt_c in enumerate(XTK):
                qk_proj_item(0, "wk", bkt, KT, xt_c, c, pp0, "pj0")

        # preset the V ones-columns
        for sc in range(KTILES):
            nc.vector.memset(V[sc][:, :, HEAD_DIM : HEAD_DIM + 1], 1.0)
        # x_v loads: DMA + Pool casts emitted now; consumed by filler
        XBV = [
            load_chunk(vd, c * 512, 4, nc.gpsimd.tensor_copy, f"v{c}") for c in range(S // 512)
        ]

        # ============ attention with projection filler ======================
        with (
            tc.tile_pool(name="ehpool", bufs=2) as ehpool,
            tc.tile_pool(name="oupool", bufs=2) as oupool,
            tc.tile_pool(name="rrpool", bufs=2) as rrpool,
            tc.tile_pool(name="rsbpool", bufs=1) as rsbpool,
            tc.tile_pool(name="ps_o", bufs=1, space="PSUM") as ps_o,
            tc.tile_pool(name="pp", bufs=1, space="PSUM") as pp,
            tc.tile_pool(name="pt", bufs=1, space="PSUM") as pt_pool,
        ):
            KB = 2
            pe_chain[0] = None  # head-0 scores may overlap the startup tail

            # ---- filler queue: V-projection, pairs 1-3 Q/K projections, and
            # the pair 0-2 out-projection partials.  Entries are
            # (min_head, closure): a closure may only be emitted at head >=
            # min_head (its tile dependencies exist by then).  Keeping the PE
            # fed with dependency-free filler through ALL heads holds the
            # Tensor clock at full speed (stalls drop it 2x and it stays
            # dropped).
            filler = []
            vt_state = [None]

            def vt_item(c, st):
                def run():
                    if st == 0:
                        vt_state[0] = xtv_pool.tile(
                            [128, 4, D_MODEL], BF16, name=f"xtv_{c}", tag="xtv"
                        )
                    xtv = vt_state[0]
                    pt = pt_pool.tile([128, D_MODEL], BF16, name="ptv", tag="ptv")
                    transpose_group(pt, XBV[c], st)
                    nc.vector.tensor_copy(
                        xtv[:, :, st * 128 : (st + 1) * 128],
                        pt.rearrange("p (c f) -> p c f", c=4),
                    )
                return run

            def vp_item(c, st):
                def run():
                    xtv = vt_state[0]
                    sc = c * 4 + st
                    pj = pp.tile([128, 512], F32, name="pjv", tag="ppj")
                    for dc in range(4):
                        chain(
                            nc.tensor.matmul(
                                pj,
                                xtv[:, dc, st * 128 : (st + 1) * 128],
                                WT["wv"][dc],
                                start=(dc == 0),
                                stop=(dc == 3),
                            )
                        )
                    pj3 = pj.rearrange("p (h d) -> p h d", h=NUM_HEADS)
                    nc.vector.tensor_add(
                        V[sc][:, :, 0:HEAD_DIM],
                        pj3,
                        bvb.rearrange("p (h d) -> p h d", h=NUM_HEADS),
                    )
                return run

            def qk_item(t, wname, bt, dst, xt_c, c):
                def run():
                    qk_proj_item(t, wname, bt, dst, xt_c, c, pp, "ppj")
                return run

            def op_item(t, sq):
                def run():
                    pj = pp.tile([128, D_MODEL], F32, name="pjo", tag="ppj")
                    chain(
                        nc.tensor.matmul(
                            pj,
                            OMT[t][:, sq * 128 : (sq + 1) * 128],
                            WT["wo"][t],
                            start=True,
                            stop=True,
                        )
                    )
                    if t == 0:
                        nc.vector.tensor_add(OACC[sq], pj, bob)
                    else:
                        nc.vector.tensor_add(OACC[sq], OACC[sq], pj)
                return run

            for c in range(S // 512):
                for st in range(4):
                    filler.append((0, vt_item(c, st)))
                    filler.append((0, vp_item(c, st)))
            for t in range(1, H2):
                mh = t - 1  # pair t's projection may start at head t-1
                for c, xt_c in enumerate(XTQ):
                    filler.append((mh, qk_item(t, "wq", bqt, QT, xt_c, c)))
                for c, xt_c in enumerate(XTK):
                    filler.append((mh, qk_item(t, "wk", bkt, KT, xt_c, c)))
            # out-projection partials: OMT[t] exists after epilogue_pair(t),
            # emitted at the end of head 2t+2 -> gate at head 2t+3.
            for t in range(H2 - 1):
                for sq in range(SQ // 128):
                    filler.append((2 * t + 4 + (sq // 4), op_item(t, sq)))

            # pops per kb batch, by head
            POPS = [3, 3, 1, 1, 1, 1, 1, 1]

            OU = {}
            RR = {}
            PO = {}
            deferred = []

            def emit_av_batch(ph, peh, kb):
                if kb == 0:
                    PO[ph] = ps_o.tile([HEAD_DIM + 1, SQ], F32, name=f"po_{ph}", tag="po")
                ppo = PO[ph]
                for j in range(KB):
                    kc = KB * kb + j
                    for qc in range(QC):
                        sl = slice(qc * 512, (qc + 1) * 512)
                        chain(
                            nc.tensor.matmul(
                                ppo[:, sl],
                                V[kc][:, ph, :],
                                peh[:, kc, sl],
                                start=(kc == 0),
                                stop=(kc == KTILES - 1),
                            )
                        )

            def epilogue_recip(t):
                # reciprocal runs in SBUF (holds no PSUM; a DVE reciprocal is
                # ~7 cyc/elem = ~7us per pair).  The PE-side broadcast is
                # DEFERRED one head: the in-order PE queue would stall behind
                # a matmul whose rcp input is still being computed.
                rcp = rsbpool.tile([33, SQ], BF16, name=f"rcp_{t}", tag="rsb")
                with nc.allow_low_precision(reason="bf16 softmax denominators, ~0.4% on a 2e-2 budget"):
                    nc.vector.reciprocal(rcp, RR[t])
                deferred.append((2 * t + 4, t, rcp))  # pop at head 2t+4: two heads after emission

            def epilogue_bcast(t, rcp):
                pr = ps_s.tile([128, SQ], F32, name=f"pr_{t}", tag="pscore")
                for qc in range(QC):
                    sl = slice(qc * 512, (qc + 1) * 512)
                    chain(nc.tensor.matmul(pr[:, sl], e2, rcp[:, sl], start=True, stop=True))
                nc.vector.tensor_mul(OMT[t], OU[t], pr)

            def finalize(ph):
                ppo = PO.pop(ph)
                t, half = ph // 2, ph % 2
                if half == 0:
                    OU[t] = oupool.tile([128, SQ], BF16, name=f"ou_{t}", tag="ou")
                    RR[t] = rrpool.tile([33, SQ], BF16, name=f"rrp_{t}", tag="rr")
                    nc.vector.memset(RR[t], 1.0)
                nc.vector.tensor_copy(
                    OU[t][half * HEAD_DIM : (half + 1) * HEAD_DIM, :],
                    ppo[0:HEAD_DIM, :],
                )
                nc.vector.tensor_copy(
                    RR[t][32 * half : 32 * half + 1, :],
                    ppo[HEAD_DIM : HEAD_DIM + 1, :],
                )
                if half == 1:
                    epilogue_recip(t)

            prev = None
            for h in range(NUM_HEADS):
                eh = ehpool.tile([128, KTILES, SQ], BF16, name=f"eh_{h}", tag="eh")
                t, half = h // 2, h % 2
                while deferred and deferred[0][0] <= h:
                    _, dt_, drcp = deferred.pop(0)
                    epilogue_bcast(dt_, drcp)
                for kb in range(KTILES // KB):
                    pss = []
                    for j in range(KB):
                        kc = KB * kb + j
                        pscore = ps_s.tile([128, SQ], F32, name="psc", tag="pscore")
                        for qc in range(QC):
                            sl = slice(qc * 512, (qc + 1) * 512)
                            chain(
                                nc.tensor.matmul(
                                    pscore[:, sl],
                                    KT[t][
                                        half * HEAD_DIM : (half + 1) * HEAD_DIM,
                                        kc * 128 : (kc + 1) * 128,
                                    ],
                                    QT[t][
                                        half * HEAD_DIM : (half + 1) * HEAD_DIM, sl
                                    ],
                                    start=True,
                                    stop=True,
                                )
                            )
                        pss.append((kc, pscore))
                    for kc, pscore in pss:
                        nc.scalar.activation(
                            eh[:, kc, :],
                            pscore,
                            mybir.ActivationFunctionType.Exp,
                            scale=SCALE,
                        )
                    if prev is not None:
                        emit_av_batch(prev[0], prev[1], kb)
                    for _ in range(POPS[h]):
                        if filler and filler[0][0] <= h:
                            filler.pop(0)[1]()
                if prev is not None:
                    finalize(prev[0])
                prev = (h, eh)
            # pair-2's broadcast (reciprocal emitted back at head 6) must
            # precede the op2 filler items below
            while deferred and deferred[0][0] <= NUM_HEADS:
                _, dt_, drcp = deferred.pop(0)
                epilogue_bcast(dt_, drcp)
            for kb in range(KTILES // KB):
                emit_av_batch(prev[0], prev[1], kb)
                if filler and filler[0][0] <= NUM_HEADS:
                    filler.pop(0)[1]()
            finalize(prev[0])
            while deferred:
                _, dt_, drcp = deferred.pop(0)
                epilogue_bcast(dt_, drcp)
            while filler:
                filler.pop(0)[1]()

            # tail: pair 3's out-projection contribution + store
            with tc.tile_pool(name="outsb", bufs=2) as outsb:
                for sq in range(SQ // 128):
                    pj = pp.tile([128, D_MODEL], F32, name="pjt", tag="ppj")
                    chain(
                        nc.tensor.matmul(
                            pj,
                            OMT[3][:, sq * 128 : (sq + 1) * 128],
                            WT["wo"][3],
                            start=True,
                            stop=True,
                        )
                    )
                    ot = outsb.tile([128, D_MODEL], F32, name="ot", tag="ot")
                    nc.vector.tensor_add(ot, pj, OACC[sq])
                    nc.sync.dma_start(out=outd[sq * 128 : (sq + 1) * 128, :], in_=ot)

    split_waits(nc)
    return nc


_cached_nc = None


def _get_nc():
    global _cached_nc
    if _cached_nc is None:
        _cached_nc = build_mha()
    return _cached_nc


def kernel(q, k, v, mask, Wq, bq, Wk, bk, Wv, bv, Wo, bo, **_unused):
    q = np.asarray(q, dtype=np.float32)
    k = np.asarray(k, dtype=np.float32)
    v = np.asarray(v, dtype=np.float32)
    weights = {
        "wq": np.ascontiguousarray(np.asarray(Wq, np.float32)),
        "wk": np.ascontiguousarray(np.asarray(Wk, np.float32)),
        "wv": np.ascontiguousarray(np.asarray(Wv, np.float32)),
        "wo": np.ascontiguousarray(np.asarray(Wo, np.float32)),
        "bq": np.ascontiguousarray(np.asarray(bq, np.float32)),
        "bk": np.ascontiguousarray(np.asarray(bk, np.float32)),
        "bv": np.ascontiguousarray(np.asarray(bv, np.float32)),
        "bo": np.ascontiguousarray(np.asarray(bo, np.float32)),
    }
    in_maps = []
    for core in range(N_CORES):
        b, qh = core // 2, core % 2
        in_maps.append(
            {
                "q": np.ascontiguousarray(q[b, qh * SQ : (qh + 1) * SQ, :]),
                "k": np.ascontiguousarray(k[b]),
                "v": np.ascontiguousarray(v[b]),
                **weights,
            }
        )
    nc = _get_nc()
    res = run_bass_kernel_spmd(nc, in_maps, list(range(N_CORES)))
    out = np.empty((B, S, D_MODEL), dtype=np.float32)
    for core in range(N_CORES):
        b, qh = core // 2, core % 2
        out[b, qh * SQ : (qh + 1) * SQ, :] = res.results[core]["out"]
    return out
